# revision 22
# baseline (speedup 1.0000x reference)
"""Bass/Tile kernel for nn_Diffeo: horizontal bilinear remap as banded matmul.

v4: fp16 end-to-end (image, weights, |t| tile, output) -> half the HBM
traffic of v3 and DVE 4x / ACT 2x fast paths; uniform 192-wide stacked
weight windows with the per-block -128k shift baked into the host-side
xh piece (so one Abs covers all 4 blocks, no per-block bias); main
matmuls use split start=True pieces so no full-width zeroing pass is
needed (tile-granular WAW deps keep PE program order); output stored
plane-major in DRAM for 4KB-contiguous DMA runs, quad-packed SBUF tiles.

Per core (H-sharded, 64 rows y, all 192 b*c planes):
  t[p, (k,x)] = (xh'-128k) + xm - p      (K=3 fp16 matmul, ~2^-13 exact)
  a           = |t|                      (ACT Abs cols [0,C), DVE abs_max rest)
  wneg        = min(a - 1, 0) = -hat     (DVE tensor_scalar, fp16 4x)
  psum_o     += D_k.T @ wneg             (PE fp16, split-start windows)
  out_sb      = -psum_o                  (ACT Copy scale=-1, 2x, fp16 out)
"""

import sys
from contextlib import ExitStack

sys.path.insert(0, "/opt/trn_rl_repo")

import numpy as np

import concourse.bass as bass
import concourse.mybir as mybir
import concourse.tile as tile
from concourse import bacc
from concourse._compat import axon_active

F32 = mybir.dt.float32
F16 = mybir.dt.float16

H = W = 512
NPLANE = 192            # 64 batches * 3 channels
NCORES = 8
YPC = H // NCORES       # 64 rows per core
YG = 8                  # rows per input-DMA group
NG = YPC // YG          # 8 groups
KBLK = 4                # x_in blocks of 128
GS = [0, 107, 235, 336] # window starts (out-x) per k block
GL = 176                # uniform window length (|dx| <= 21 both ways, asserted)
SW = KBLK * GL          # stacked window width = 704

# weight zones in the wts tile: (rhs0, rhs1, out0, start).
# k0 is full-width with a persistent zero pad (cols 176:512), so it carries
# start=True for the whole bank; k>=1 are pure accumulates (order-free).
WZONE = [
    (0, 512, 0, True),
    (512, 688, 107, False),
    (688, 864, 235, False),
    (864, 1040, 336, False),
]
WTW = 1040              # wts tile width


def build_program(num_devices: int = NCORES):
    nc = bacc.Bacc(
        "TRN2",
        target_bir_lowering=False,
        debug=not axon_active(),
        num_devices=num_devices,
    )
    imgT = nc.dram_tensor("imgT", [NG, W, YG * NPLANE], F16, kind="ExternalInput").ap()
    xn3 = nc.dram_tensor("xn3", [12, (YPC // 4) * SW], F16, kind="ExternalInput").ap()
    c4 = nc.dram_tensor("c4", [128, 128], F16, kind="ExternalInput").ap()
    # planes 0-127: [plane, y, x]; planes 128-191: [y%2, plane-128, y//2, x]
    out = nc.dram_tensor("out", [128, YPC, W], F16, kind="ExternalOutput").ap()
    out2 = nc.dram_tensor("out2", [2, 64, YPC // 2, W], F16, kind="ExternalOutput").ap()

    with tile.TileContext(nc) as tc, ExitStack() as ctx:
        const_pool = ctx.enter_context(tc.tile_pool(name="const", bufs=1))
        dpool = ctx.enter_context(tc.tile_pool(name="dt", bufs=2))
        tapool = ctx.enter_context(tc.tile_pool(name="psum_ta", bufs=2, space="PSUM"))
        tbpool = ctx.enter_context(tc.tile_pool(name="psum_tb", bufs=2, space="PSUM"))
        opool1 = ctx.enter_context(tc.tile_pool(name="psum_o1", bufs=2, space="PSUM"))
        opool2 = ctx.enter_context(tc.tile_pool(name="psum_o2", bufs=2, space="PSUM"))
        apool = ctx.enter_context(tc.tile_pool(name="abs", bufs=3))
        wpool = ctx.enter_context(tc.tile_pool(name="wts", bufs=3))
        spool1 = ctx.enter_context(tc.tile_pool(name="osb1", bufs=2))
        spool2 = ctx.enter_context(tc.tile_pool(name="osb2", bufs=2))

        NF = (YPC // 4) * SW
        xn3_sb = const_pool.tile([128, NF], F16)
        for a in range(4):
            nc.sync.dma_start(xn3_sb[32 * a : 32 * a + 3, :], xn3[3 * a : 3 * a + 3, :])
        c4_sb = const_pool.tile([128, 128], F16)
        nc.sync.dma_start(c4_sb[:], c4[:])

        # zero the persistent k0 pad of each wts ring buffer once
        for _ in range(3):
            wz = wpool.tile([128, WTW], F16, tag="w", name="wts")
            nc.vector.memset(wz[:, 176:512], 0.0)

        osb1 = osb2 = None
        for g in range(NG):
            dtk = dpool.tile([128, KBLK * YG * NPLANE], F16, tag="dt")
            nc.sync.dma_start(
                dtk[:].rearrange("p (k c) -> p k c", k=KBLK),
                imgT[g].rearrange("(k p) c -> p k c", k=KBLK),
            )
            for yy in range(0, YG, 2):
                ys = (g * YG + yy, g * YG + yy + 1)
                q0 = yy % 4
                # --- t matmuls for the pair: disjoint PE row groups
                # (32*(y%4)) start ~4ns apart and run concurrently; t lives
                # in two 1-bank tiles (A: cols 0-512, B: 512-SW) so the next
                # pair's t can start as soon as the matching abs is done ---
                ptas, ptbs, wtss = [], [], []
                for y in ys:
                    ptas.append(tapool.tile([128, 512], F32, tag="ta", name="pta"))
                    ptbs.append(
                        tbpool.tile([128, SW - 512], F32, tag="tb", name="ptb")
                    )
                for i, y in enumerate(ys):
                    a32, f = 32 * (y % 4), y // 4
                    nc.tensor.matmul(
                        ptas[i][:],
                        lhsT=c4_sb[a32 : a32 + 3, :],
                        rhs=xn3_sb[a32 : a32 + 3, f * SW : f * SW + 512],
                        start=True,
                        stop=True,
                        tile_position=(a32, 0),
                    )
                for i, y in enumerate(ys):
                    a32, f = 32 * (y % 4), y // 4
                    nc.tensor.matmul(
                        ptbs[i][:],
                        lhsT=c4_sb[a32 : a32 + 3, :],
                        rhs=xn3_sb[a32 : a32 + 3, f * SW + 512 : (f + 1) * SW],
                        start=True,
                        stop=True,
                        tile_position=(a32, 0),
                    )
                # --- a = |t| (ACT, per segment), wneg = min(a-1,0) (DVE 4x) ---
                for i in range(2):
                    at = apool.tile([128, SW], F16, tag="a", name="at")
                    nc.scalar.activation(
                        at[:, 0:512], ptas[i][:], mybir.ActivationFunctionType.Abs
                    )
                    nc.scalar.activation(
                        at[:, 512:SW], ptbs[i][:], mybir.ActivationFunctionType.Abs
                    )
                    wts = wpool.tile([128, WTW], F16, tag="w", name="wts")
                    nc.vector.tensor_scalar(
                        wts[:, 0:176],
                        at[:, 0:176],
                        1.0,
                        0.0,
                        op0=mybir.AluOpType.subtract,
                        op1=mybir.AluOpType.min,
                    )
                    nc.vector.tensor_scalar(
                        wts[:, 512:WTW],
                        at[:, 176:SW],
                        1.0,
                        0.0,
                        op0=mybir.AluOpType.subtract,
                        op1=mybir.AluOpType.min,
                    )
                    wtss.append(wts)
                # --- banded matmuls, split-start windows ---
                if q0 == 0:
                    osb1 = spool1.tile([128, 4 * W], F16, tag="o1", name="osb1")
                    osb2 = spool2.tile([128, 2 * W], F32, tag="o2", name="osb2")
                for i, y in enumerate(ys):
                    yloc = yy + i
                    # o1 split into two M=64 halves at PE col groups 0/64 so
                    # both halves stream concurrently on separate XBUSes
                    psum_o1 = opool1.tile([128, W], F32, tag="po1", name="po1")
                    for k in range(KBLK):
                        base = k * YG * NPLANE + yloc * NPLANE
                        (r0, r1, o0, st) = WZONE[k]
                        for j in range(2):
                            l1 = dtk[:, base + 64 * j : base + 64 * j + 64]
                            nc.tensor.matmul(
                                psum_o1[64 * j : 64 * j + 64, o0 : o0 + (r1 - r0)],
                                lhsT=l1,
                                rhs=wtss[i][:, r0:r1],
                                start=st,
                                stop=(k == KBLK - 1),
                                skip_group_check=True,
                                tile_position=(0, 64 * j),
                            )
                    nc.vector.tensor_scalar_mul(
                        osb1[:, (q0 + i) * W : (q0 + i + 1) * W], psum_o1[:], -1.0
                    )
                # o2: pair-interleaved at disjoint PE col groups (0 / 64) so
                # the two streams run concurrently
                psum_o2 = opool2.tile([128, W], F32, tag="po2", name="po2")
                for k in range(KBLK):
                    (r0, r1, o0, st) = WZONE[k]
                    for i in range(2):
                        base = k * YG * NPLANE + (yy + i) * NPLANE
                        l2 = dtk[:, base + 128 : base + NPLANE]
                        nc.tensor.matmul(
                            psum_o2[64 * i : 64 * i + 64, o0 : o0 + (r1 - r0)],
                            lhsT=l2,
                            rhs=wtss[i][:, r0:r1],
                            start=st,
                            stop=(k == KBLK - 1),
                            skip_group_check=True,
                            tile_position=(0, 64 * i),
                        )
                # o2 copy on ACT stays fp32 (2x path); the SWDGE output DMA
                # casts fp32 -> fp16 on the fly
                nc.scalar.mul(
                    osb2[:, (q0 // 2) * W : (q0 // 2 + 1) * W], psum_o2[:], -1.0
                )
                if q0 == 2:
                    y0 = ys[1] - 3
                    pr0 = y0 // 2
                    nc.sync.dma_start(
                        out[0:128, y0 : y0 + 4, :],
                        osb1[:].rearrange("p (y x) -> p y x", y=4),
                    )
                    nc.gpsimd.dma_start(
                        out2[:, :, pr0 : pr0 + 2, :].rearrange(
                            "j p r x -> (j p) (r x)"
                        ),
                        osb2[:],
                    )

    nc.compile()
    return nc


# ---------------- host-side helpers ----------------

def host_xn(c_u: np.ndarray) -> np.ndarray:
    """float64 reproduction of the reference displacement; returns xn [H, W] f32."""
    import math

    CUT = 16
    k = np.arange(1, CUT + 1, dtype=np.float64)
    i, j = np.meshgrid(k, k, indexing="ij")
    r = np.sqrt(i * i + j * j)
    e = (r < CUT + 0.5).astype(np.float64) / r
    x = np.linspace(0.0, 1.0, W, dtype=np.float64)
    s = np.sin(np.pi * x[:, None] * k[None, :])
    u = np.einsum("ij,xi,yj->yx", c_u.astype(np.float64) * e, s, s)
    Tw = 4.0 / (math.pi**3 * CUT**2 * math.log(CUT))
    dx = math.sqrt(Tw) * u * W
    xg = np.arange(W, dtype=np.float64)
    return np.clip(xg[None, :] - dx, 0.0, W - 1.0).astype(np.float32)


def _mask11(v: np.ndarray) -> np.ndarray:
    """Truncate fp32 mantissa to 10 explicit bits (fits fp16 significand)."""
    return (v.view(np.uint32) & np.uint32(0xFFFFE000)).view(np.float32)


def host_prep(img: np.ndarray, c_u: np.ndarray):
    """Build per-core input maps."""
    xn = host_xn(c_u)
    # band-coverage check: every tap P of column x must lie in window P//128
    m = np.floor(xn).astype(np.int64)
    xg = np.arange(W)[None, :]
    for tap in (m, np.minimum(m + 1, W - 1)):
        kk = tap // 128
        rel = xg - np.take(np.asarray(GS + [0]), kk)
        assert (rel >= 0).all() and (rel < GL).all(), "window coverage violated"
    planes = img.reshape(NPLANE, H, W)
    imgT_all = np.ascontiguousarray(
        planes.reshape(NPLANE, NCORES, NG, YG, W)
        .transpose(1, 2, 4, 3, 0)
        .astype(np.float16)
    )
    c4m = np.zeros((128, 128), np.float16)
    p = np.arange(128, dtype=np.float32)
    for a in range(4):
        c4m[32 * a + 0, :] = 1.0
        c4m[32 * a + 1, :] = 1.0
        c4m[32 * a + 2, :] = (-p).astype(np.float16)
    in_maps = []
    NF = (YPC // 4) * SW
    for core in range(NCORES):
        xns = np.zeros((12, NF), np.float16)
        for y in range(YPC):
            a, f = y % 4, y // 4
            yg = core * YPC + y
            for k in range(KBLK):
                seg = xn[yg, GS[k] : GS[k] + GL]
                xh = _mask11(seg)
                xm = (seg - xh).astype(np.float16)
                c0 = f * SW + GL * k
                xns[3 * a + 0, c0 : c0 + GL] = (xh - 128.0 * k).astype(np.float16)
                xns[3 * a + 1, c0 : c0 + GL] = xm
                xns[3 * a + 2, c0 : c0 + GL] = 1.0
        in_maps.append(
            {
                "imgT": imgT_all[core].reshape(NG, W, YG * NPLANE),
                "xn3": xns,
                "c4": c4m,
            }
        )
    return in_maps


def host_gather(outs: list) -> np.ndarray:
    """Assemble per-core 'out'/'out2' into [64, 3, H, W]."""
    full = np.empty((64, 3, H, W), np.float32)
    for core, om in enumerate(outs):
        o1 = om["out"].astype(np.float32)  # [128, YPC, W]
        # out2 [j, pl, pair, x] -> [pl, y=(pair*2+j), x]
        o2 = (
            om["out2"]
            .astype(np.float32)
            .transpose(1, 2, 0, 3)
            .reshape(64, YPC, W)
        )
        o = np.concatenate([o1, o2], axis=0)  # [NPLANE, YPC, W]
        full[:, :, core * YPC : (core + 1) * YPC, :] = o.reshape(64, 3, YPC, W)
    return full


# ---------------- harness entry point ----------------

_NC_CACHE = {}


def kernel(img: "np.ndarray", c_u: "np.ndarray", c_v: "np.ndarray") -> "np.ndarray":
    """Full-input entry: shard across 8 NeuronCores, run, reassemble."""
    img = np.ascontiguousarray(np.asarray(img, dtype=np.float32))
    c_u = np.asarray(c_u, dtype=np.float32)
    in_maps = host_prep(img, c_u)
    if "nc" not in _NC_CACHE:
        _NC_CACHE["nc"] = build_program(num_devices=NCORES)
    from concourse.bass_utils import run_bass_kernel_spmd

    res = run_bass_kernel_spmd(
        _NC_CACHE["nc"], in_maps, core_ids=list(range(NCORES)), trace=False
    )
    return host_gather(res.results)


# revision 24
# speedup vs baseline: 1.1698x; 1.1698x over previous
"""Bass/Tile kernel for nn_Diffeo: horizontal bilinear remap as banded matmul.

v4: fp16 end-to-end (image, weights, |t| tile, output) -> half the HBM
traffic of v3 and DVE 4x / ACT 2x fast paths; uniform 192-wide stacked
weight windows with the per-block -128k shift baked into the host-side
xh piece (so one Abs covers all 4 blocks, no per-block bias); main
matmuls use split start=True pieces so no full-width zeroing pass is
needed (tile-granular WAW deps keep PE program order); output stored
plane-major in DRAM for 4KB-contiguous DMA runs, quad-packed SBUF tiles.

Per core (H-sharded, 64 rows y, all 192 b*c planes):
  t[p, (k,x)] = (xh'-128k) + xm - p      (K=3 fp16 matmul, ~2^-13 exact)
  a           = |t|                      (ACT Abs cols [0,C), DVE abs_max rest)
  wneg        = min(a - 1, 0) = -hat     (DVE tensor_scalar, fp16 4x)
  psum_o     += D_k.T @ wneg             (PE fp16, split-start windows)
  out_sb      = -psum_o                  (ACT Copy scale=-1, 2x, fp16 out)
"""

import sys
from contextlib import ExitStack

sys.path.insert(0, "/opt/trn_rl_repo")

import numpy as np

import concourse.bass as bass
import concourse.mybir as mybir
import concourse.tile as tile
from concourse import bacc
from concourse._compat import axon_active

F32 = mybir.dt.float32
F16 = mybir.dt.float16

H = W = 512
NPLANE = 192            # 64 batches * 3 channels
NCORES = 8
YPC = H // NCORES       # 64 rows per core
YG = 8                  # rows per input-DMA group
NG = YPC // YG          # 8 groups
KBLK = 4                # x_in blocks of 128
GS = [0, 107, 235, 336] # window starts (out-x) per k block
GL = 176                # uniform window length (|dx| <= 21 both ways, asserted)
SW = KBLK * GL          # stacked window width = 704

# split pieces per k: (c0, c1, out0, start) relative to the window;
# each start=False piece overlaps the preceding start=True piece's region,
# so tile-granular WAW deps preserve the required order.
PIECES = [
    [(0, 176, 0, True)],
    [(0, 69, 107, False), (69, 176, 176, True)],
    [(0, 48, 235, False), (48, 176, 283, True)],
    [(0, 75, 336, False), (75, 176, 411, True)],
]


def build_program(num_devices: int = NCORES):
    nc = bacc.Bacc(
        "TRN2",
        target_bir_lowering=False,
        debug=not axon_active(),
        num_devices=num_devices,
    )
    imgT = nc.dram_tensor("imgT", [NG, W, YG * NPLANE], F16, kind="ExternalInput").ap()
    xn3 = nc.dram_tensor("xn3", [12, (YPC // 4) * SW], F16, kind="ExternalInput").ap()
    c4 = nc.dram_tensor("c4", [128, 128], F16, kind="ExternalInput").ap()
    # planes 0-127: [plane, y, x]; planes 128-191: [y%2, plane-128, y//2, x]
    out = nc.dram_tensor("out", [128, YPC, W], F16, kind="ExternalOutput").ap()
    out2 = nc.dram_tensor("out2", [2, 64, YPC // 2, W], F16, kind="ExternalOutput").ap()

    with tile.TileContext(nc) as tc, ExitStack() as ctx:
        const_pool = ctx.enter_context(tc.tile_pool(name="const", bufs=1))
        dpool = ctx.enter_context(tc.tile_pool(name="dt", bufs=3))
        tapool = ctx.enter_context(tc.tile_pool(name="psum_ta", bufs=2, space="PSUM"))
        tbpool = ctx.enter_context(tc.tile_pool(name="psum_tb", bufs=2, space="PSUM"))
        opool1 = ctx.enter_context(tc.tile_pool(name="psum_o1", bufs=2, space="PSUM"))
        opool2 = ctx.enter_context(tc.tile_pool(name="psum_o2", bufs=2, space="PSUM"))
        apool = ctx.enter_context(tc.tile_pool(name="abs", bufs=4))
        wpool = ctx.enter_context(tc.tile_pool(name="wts", bufs=4))
        spool1 = ctx.enter_context(tc.tile_pool(name="osb1", bufs=3))
        spool2 = ctx.enter_context(tc.tile_pool(name="osb2", bufs=3))

        NF = (YPC // 4) * SW
        xn3_sb = const_pool.tile([128, NF], F16)
        for a in range(4):
            nc.sync.dma_start(xn3_sb[32 * a : 32 * a + 3, :], xn3[3 * a : 3 * a + 3, :])
        c4_sb = const_pool.tile([128, 128], F16)
        nc.sync.dma_start(c4_sb[:], c4[:])

        osb1 = osb2 = None
        for g in range(NG):
            dtk = dpool.tile([128, KBLK * YG * NPLANE], F16, tag="dt")
            nc.sync.dma_start(
                dtk[:].rearrange("p (k c) -> p k c", k=KBLK),
                imgT[g].rearrange("(k p) c -> p k c", k=KBLK),
            )
            for yy in range(0, YG, 2):
                ys = (g * YG + yy, g * YG + yy + 1)
                q0 = yy % 4
                # --- t matmuls for the pair: disjoint PE row groups
                # (32*(y%4)) start ~4ns apart and run concurrently; t lives
                # in two 1-bank tiles (A: cols 0-512, B: 512-SW) so the next
                # pair's t can start as soon as the matching abs is done ---
                ptas, ptbs, wtss = [], [], []
                for y in ys:
                    ptas.append(tapool.tile([128, 512], F32, tag="ta", name="pta"))
                    ptbs.append(
                        tbpool.tile([128, SW - 512], F32, tag="tb", name="ptb")
                    )
                for i, y in enumerate(ys):
                    a32, f = 32 * (y % 4), y // 4
                    nc.tensor.matmul(
                        ptas[i][:],
                        lhsT=c4_sb[a32 : a32 + 3, :],
                        rhs=xn3_sb[a32 : a32 + 3, f * SW : f * SW + 512],
                        start=True,
                        stop=True,
                        tile_position=(a32, 0),
                    )
                for i, y in enumerate(ys):
                    a32, f = 32 * (y % 4), y // 4
                    nc.tensor.matmul(
                        ptbs[i][:],
                        lhsT=c4_sb[a32 : a32 + 3, :],
                        rhs=xn3_sb[a32 : a32 + 3, f * SW + 512 : (f + 1) * SW],
                        start=True,
                        stop=True,
                        tile_position=(a32, 0),
                    )
                # --- a = |t| (ACT, per segment), wneg = min(a-1,0) (DVE 4x) ---
                for i in range(2):
                    at = apool.tile([128, SW], F16, tag="a", name="at")
                    nc.scalar.activation(
                        at[:, 0:512], ptas[i][:], mybir.ActivationFunctionType.Abs
                    )
                    nc.scalar.activation(
                        at[:, 512:SW], ptbs[i][:], mybir.ActivationFunctionType.Abs
                    )
                    wts = wpool.tile([128, SW], F16, tag="w", name="wts")
                    nc.vector.tensor_scalar(
                        wts[:],
                        at[:],
                        1.0,
                        0.0,
                        op0=mybir.AluOpType.subtract,
                        op1=mybir.AluOpType.min,
                    )
                    wtss.append(wts)
                # --- banded matmuls, split-start windows ---
                if q0 == 0:
                    osb1 = spool1.tile([128, 4 * W], F16, tag="o1", name="osb1")
                    osb2 = spool2.tile([128, 2 * W], F32, tag="o2", name="osb2")
                for i, y in enumerate(ys):
                    yloc = yy + i
                    # o1 split into two M=64 halves at PE col groups 0/64 so
                    # both halves stream concurrently on separate XBUSes
                    psum_o1 = opool1.tile([128, W], F32, tag="po1", name="po1")
                    for k in range(KBLK):
                        base = k * YG * NPLANE + yloc * NPLANE
                        for j in range(2):
                            l1 = dtk[:, base + 64 * j : base + 64 * j + 64]
                            for (c0, c1, o0, st) in PIECES[k]:
                                nc.tensor.matmul(
                                    psum_o1[64 * j : 64 * j + 64, o0 : o0 + (c1 - c0)],
                                    lhsT=l1,
                                    rhs=wtss[i][:, GL * k + c0 : GL * k + c1],
                                    start=st,
                                    stop=(k == KBLK - 1 and c1 == GL),
                                    skip_group_check=True,
                                    tile_position=(0, 64 * j),
                                )
                    nc.vector.tensor_scalar_mul(
                        osb1[:, (q0 + i) * W : (q0 + i + 1) * W], psum_o1[:], -1.0
                    )
                # o2: pair-interleaved at disjoint PE col groups (0 / 64) so
                # the two streams run concurrently
                psum_o2 = opool2.tile([128, W], F32, tag="po2", name="po2")
                for k in range(KBLK):
                    for i in range(2):
                        base = k * YG * NPLANE + (yy + i) * NPLANE
                        l2 = dtk[:, base + 128 : base + NPLANE]
                        for (c0, c1, o0, st) in PIECES[k]:
                            nc.tensor.matmul(
                                psum_o2[64 * i : 64 * i + 64, o0 : o0 + (c1 - c0)],
                                lhsT=l2,
                                rhs=wtss[i][:, GL * k + c0 : GL * k + c1],
                                start=st,
                                stop=(k == KBLK - 1 and c1 == GL),
                                skip_group_check=True,
                                tile_position=(0, 64 * i),
                            )
                # o2 copy on ACT stays fp32 (2x path); the SWDGE output DMA
                # casts fp32 -> fp16 on the fly
                nc.scalar.mul(
                    osb2[:, (q0 // 2) * W : (q0 // 2 + 1) * W], psum_o2[:], -1.0
                )
                if q0 == 2:
                    y0 = ys[1] - 3
                    pr0 = y0 // 2
                    nc.sync.dma_start(
                        out[0:128, y0 : y0 + 4, :],
                        osb1[:].rearrange("p (y x) -> p y x", y=4),
                    )
                    nc.gpsimd.dma_start(
                        out2[:, :, pr0 : pr0 + 2, :].rearrange(
                            "j p r x -> (j p) (r x)"
                        ),
                        osb2[:],
                    )

    nc.compile()
    return nc


# ---------------- host-side helpers ----------------

def host_xn(c_u: np.ndarray) -> np.ndarray:
    """float64 reproduction of the reference displacement; returns xn [H, W] f32."""
    import math

    CUT = 16
    k = np.arange(1, CUT + 1, dtype=np.float64)
    i, j = np.meshgrid(k, k, indexing="ij")
    r = np.sqrt(i * i + j * j)
    e = (r < CUT + 0.5).astype(np.float64) / r
    x = np.linspace(0.0, 1.0, W, dtype=np.float64)
    s = np.sin(np.pi * x[:, None] * k[None, :])
    u = np.einsum("ij,xi,yj->yx", c_u.astype(np.float64) * e, s, s)
    Tw = 4.0 / (math.pi**3 * CUT**2 * math.log(CUT))
    dx = math.sqrt(Tw) * u * W
    xg = np.arange(W, dtype=np.float64)
    return np.clip(xg[None, :] - dx, 0.0, W - 1.0).astype(np.float32)


def _mask11(v: np.ndarray) -> np.ndarray:
    """Truncate fp32 mantissa to 10 explicit bits (fits fp16 significand)."""
    return (v.view(np.uint32) & np.uint32(0xFFFFE000)).view(np.float32)


def host_prep(img: np.ndarray, c_u: np.ndarray):
    """Build per-core input maps."""
    xn = host_xn(c_u)
    # band-coverage check: every tap P of column x must lie in window P//128
    m = np.floor(xn).astype(np.int64)
    xg = np.arange(W)[None, :]
    for tap in (m, np.minimum(m + 1, W - 1)):
        kk = tap // 128
        rel = xg - np.take(np.asarray(GS + [0]), kk)
        assert (rel >= 0).all() and (rel < GL).all(), "window coverage violated"
    planes = img.reshape(NPLANE, H, W)
    imgT_all = np.ascontiguousarray(
        planes.reshape(NPLANE, NCORES, NG, YG, W)
        .transpose(1, 2, 4, 3, 0)
        .astype(np.float16)
    )
    c4m = np.zeros((128, 128), np.float16)
    p = np.arange(128, dtype=np.float32)
    for a in range(4):
        c4m[32 * a + 0, :] = 1.0
        c4m[32 * a + 1, :] = 1.0
        c4m[32 * a + 2, :] = (-p).astype(np.float16)
    in_maps = []
    NF = (YPC // 4) * SW
    for core in range(NCORES):
        xns = np.zeros((12, NF), np.float16)
        for y in range(YPC):
            a, f = y % 4, y // 4
            yg = core * YPC + y
            for k in range(KBLK):
                seg = xn[yg, GS[k] : GS[k] + GL]
                xh = _mask11(seg)
                xm = (seg - xh).astype(np.float16)
                c0 = f * SW + GL * k
                xns[3 * a + 0, c0 : c0 + GL] = (xh - 128.0 * k).astype(np.float16)
                xns[3 * a + 1, c0 : c0 + GL] = xm
                xns[3 * a + 2, c0 : c0 + GL] = 1.0
        in_maps.append(
            {
                "imgT": imgT_all[core].reshape(NG, W, YG * NPLANE),
                "xn3": xns,
                "c4": c4m,
            }
        )
    return in_maps


def host_gather(outs: list) -> np.ndarray:
    """Assemble per-core 'out'/'out2' into [64, 3, H, W]."""
    full = np.empty((64, 3, H, W), np.float32)
    for core, om in enumerate(outs):
        o1 = om["out"].astype(np.float32)  # [128, YPC, W]
        # out2 [j, pl, pair, x] -> [pl, y=(pair*2+j), x]
        o2 = (
            om["out2"]
            .astype(np.float32)
            .transpose(1, 2, 0, 3)
            .reshape(64, YPC, W)
        )
        o = np.concatenate([o1, o2], axis=0)  # [NPLANE, YPC, W]
        full[:, :, core * YPC : (core + 1) * YPC, :] = o.reshape(64, 3, YPC, W)
    return full


# ---------------- harness entry point ----------------

_NC_CACHE = {}


def kernel(img: "np.ndarray", c_u: "np.ndarray", c_v: "np.ndarray") -> "np.ndarray":
    """Full-input entry: shard across 8 NeuronCores, run, reassemble."""
    img = np.ascontiguousarray(np.asarray(img, dtype=np.float32))
    c_u = np.asarray(c_u, dtype=np.float32)
    in_maps = host_prep(img, c_u)
    if "nc" not in _NC_CACHE:
        _NC_CACHE["nc"] = build_program(num_devices=NCORES)
    from concourse.bass_utils import run_bass_kernel_spmd

    res = run_bass_kernel_spmd(
        _NC_CACHE["nc"], in_maps, core_ids=list(range(NCORES)), trace=False
    )
    return host_gather(res.results)


# revision 25
# speedup vs baseline: 1.2720x; 1.0873x over previous
"""Bass/Tile kernel for nn_Diffeo: horizontal bilinear remap as banded matmul.

v4: fp16 end-to-end (image, weights, |t| tile, output) -> half the HBM
traffic of v3 and DVE 4x / ACT 2x fast paths; uniform 192-wide stacked
weight windows with the per-block -128k shift baked into the host-side
xh piece (so one Abs covers all 4 blocks, no per-block bias); main
matmuls use split start=True pieces so no full-width zeroing pass is
needed (tile-granular WAW deps keep PE program order); output stored
plane-major in DRAM for 4KB-contiguous DMA runs, quad-packed SBUF tiles.

Per core (H-sharded, 64 rows y, all 192 b*c planes):
  t[p, (k,x)] = (xh'-128k) + xm - p      (K=3 fp16 matmul, ~2^-13 exact)
  a           = |t|                      (ACT Abs cols [0,C), DVE abs_max rest)
  wneg        = min(a - 1, 0) = -hat     (DVE tensor_scalar, fp16 4x)
  psum_o     += D_k.T @ wneg             (PE fp16, split-start windows)
  out_sb      = -psum_o                  (ACT Copy scale=-1, 2x, fp16 out)
"""

import sys
from contextlib import ExitStack

sys.path.insert(0, "/opt/trn_rl_repo")

import numpy as np

import concourse.bass as bass
import concourse.mybir as mybir
import concourse.tile as tile
from concourse import bacc
from concourse._compat import axon_active

F32 = mybir.dt.float32
F16 = mybir.dt.float16

H = W = 512
NPLANE = 192            # 64 batches * 3 channels
NCORES = 8
YPC = H // NCORES       # 64 rows per core
YG = 8                  # rows per input-DMA group
NG = YPC // YG          # 8 groups
KBLK = 4                # x_in blocks of 128
GS = [0, 107, 235, 336] # window starts (out-x) per k block
GL = 176                # uniform window length (|dx| <= 21 both ways, asserted)
SW = KBLK * GL          # stacked window width = 704

# split pieces per k: (c0, c1, out0, start) relative to the window;
# each start=False piece overlaps the preceding start=True piece's region,
# so tile-granular WAW deps preserve the required order.
PIECES = [
    [(0, 176, 0, True)],
    [(0, 69, 107, False), (69, 176, 176, True)],
    [(0, 48, 235, False), (48, 176, 283, True)],
    [(0, 75, 336, False), (75, 176, 411, True)],
]


def build_program(num_devices: int = NCORES):
    nc = bacc.Bacc(
        "TRN2",
        target_bir_lowering=False,
        debug=not axon_active(),
        num_devices=num_devices,
    )
    imgT = nc.dram_tensor("imgT", [NG, W, YG * NPLANE], F16, kind="ExternalInput").ap()
    xn3 = nc.dram_tensor("xn3", [12, (YPC // 4) * SW], F16, kind="ExternalInput").ap()
    c4 = nc.dram_tensor("c4", [128, 128], F16, kind="ExternalInput").ap()
    # planes 0-127: [plane, y, x]; planes 128-191: [y%2, plane-128, y//2, x]
    out = nc.dram_tensor("out", [128, YPC, W], F16, kind="ExternalOutput").ap()
    out2 = nc.dram_tensor("out2", [2, 64, YPC // 2, W], F16, kind="ExternalOutput").ap()

    with tile.TileContext(nc) as tc, ExitStack() as ctx:
        const_pool = ctx.enter_context(tc.tile_pool(name="const", bufs=1))
        dpool = ctx.enter_context(tc.tile_pool(name="dt", bufs=3))
        tapool = ctx.enter_context(tc.tile_pool(name="psum_ta", bufs=2, space="PSUM"))
        tbpool = ctx.enter_context(tc.tile_pool(name="psum_tb", bufs=2, space="PSUM"))
        opool1 = ctx.enter_context(tc.tile_pool(name="psum_o1", bufs=2, space="PSUM"))
        opool2 = ctx.enter_context(tc.tile_pool(name="psum_o2", bufs=2, space="PSUM"))
        apool = ctx.enter_context(tc.tile_pool(name="abs", bufs=4))
        wpool = ctx.enter_context(tc.tile_pool(name="wts", bufs=4))
        spool1 = ctx.enter_context(tc.tile_pool(name="osb1", bufs=3))
        spool2 = ctx.enter_context(tc.tile_pool(name="osb2", bufs=3))

        NF = (YPC // 4) * SW
        xn3_sb = const_pool.tile([128, NF], F16)
        for a in range(4):
            nc.sync.dma_start(xn3_sb[32 * a : 32 * a + 3, :], xn3[3 * a : 3 * a + 3, :])
        c4_sb = const_pool.tile([128, 128], F16)
        nc.sync.dma_start(c4_sb[:], c4[:])

        osb1 = osb2 = None
        for g in range(NG):
            dtk = dpool.tile([128, KBLK * YG * NPLANE], F16, tag="dt")
            nc.sync.dma_start(
                dtk[:].rearrange("p (k c) -> p k c", k=KBLK),
                imgT[g].rearrange("(k p) c -> p k c", k=KBLK),
            )
            for yy in range(0, YG, 2):
                ys = (g * YG + yy, g * YG + yy + 1)
                q0 = yy % 4
                # --- t matmuls for the pair: disjoint PE row groups
                # (32*(y%4)) start ~4ns apart and run concurrently; t lives
                # in two 1-bank tiles (A: cols 0-512, B: 512-SW) so the next
                # pair's t can start as soon as the matching abs is done ---
                ptas, ptbs, wtss = [], [], []
                for y in ys:
                    ptas.append(tapool.tile([128, 512], F32, tag="ta", name="pta"))
                    ptbs.append(
                        tbpool.tile([128, SW - 512], F32, tag="tb", name="ptb")
                    )
                for i, y in enumerate(ys):
                    a32, f = 32 * (y % 4), y // 4
                    nc.tensor.matmul(
                        ptas[i][:],
                        lhsT=c4_sb[a32 : a32 + 3, :],
                        rhs=xn3_sb[a32 : a32 + 3, f * SW : f * SW + 512],
                        start=True,
                        stop=True,
                        tile_position=(a32, 0),
                    )
                for i, y in enumerate(ys):
                    a32, f = 32 * (y % 4), y // 4
                    nc.tensor.matmul(
                        ptbs[i][:],
                        lhsT=c4_sb[a32 : a32 + 3, :],
                        rhs=xn3_sb[a32 : a32 + 3, f * SW + 512 : (f + 1) * SW],
                        start=True,
                        stop=True,
                        tile_position=(a32, 0),
                    )
                # --- a = |t| (ACT, per segment), wneg = min(a-1,0) (DVE 4x) ---
                for i in range(2):
                    at = apool.tile([128, SW], F16, tag="a", name="at")
                    nc.scalar.activation(
                        at[:, 0:512], ptas[i][:], mybir.ActivationFunctionType.Abs
                    )
                    nc.scalar.activation(
                        at[:, 512:SW], ptbs[i][:], mybir.ActivationFunctionType.Abs
                    )
                    wts = wpool.tile([128, SW], F16, tag="w", name="wts")
                    nc.vector.tensor_scalar(
                        wts[:],
                        at[:],
                        1.0,
                        0.0,
                        op0=mybir.AluOpType.subtract,
                        op1=mybir.AluOpType.min,
                    )
                    wtss.append(wts)
                # --- banded matmuls, split-start windows ---
                if q0 == 0:
                    osb1 = spool1.tile([128, 4 * W], F16, tag="o1", name="osb1")
                    osb2 = spool2.tile([128, 2 * W], F32, tag="o2", name="osb2")
                for i, y in enumerate(ys):
                    yloc = yy + i
                    # o1 split into two M=64 halves at PE col groups 0/64 so
                    # both halves stream concurrently on separate XBUSes
                    psum_o1 = opool1.tile([128, W], F32, tag="po1", name="po1")
                    for k in range(KBLK):
                        base = k * YG * NPLANE + yloc * NPLANE
                        for j in range(2):
                            l1 = dtk[:, base + 64 * j : base + 64 * j + 64]
                            for (c0, c1, o0, st) in PIECES[k]:
                                nc.tensor.matmul(
                                    psum_o1[64 * j : 64 * j + 64, o0 : o0 + (c1 - c0)],
                                    lhsT=l1,
                                    rhs=wtss[i][:, GL * k + c0 : GL * k + c1],
                                    start=st,
                                    stop=(k == KBLK - 1 and c1 == GL),
                                    skip_group_check=True,
                                    tile_position=(0, 64 * j),
                                )
                    nc.vector.tensor_scalar_mul(
                        osb1[:, (q0 + i) * W : (q0 + i + 1) * W], psum_o1[:], -1.0
                    )
                # o2: pair-interleaved at disjoint PE col groups (0 / 64) so
                # the two streams run concurrently
                psum_o2 = opool2.tile([128, W], F32, tag="po2", name="po2")
                for k in range(KBLK):
                    for i in range(2):
                        base = k * YG * NPLANE + (yy + i) * NPLANE
                        l2 = dtk[:, base + 128 : base + NPLANE]
                        for (c0, c1, o0, st) in PIECES[k]:
                            nc.tensor.matmul(
                                psum_o2[64 * i : 64 * i + 64, o0 : o0 + (c1 - c0)],
                                lhsT=l2,
                                rhs=wtss[i][:, GL * k + c0 : GL * k + c1],
                                start=st,
                                stop=(k == KBLK - 1 and c1 == GL),
                                skip_group_check=True,
                                tile_position=(0, 64 * i),
                            )
                # o2 copy stays fp32 (SWDGE output DMA casts to fp16);
                # alternate it between ACT and DVE per pair to balance the
                # two queues and keep ACT's FIFO clear for the abs pass
                o2dst = osb2[:, (q0 // 2) * W : (q0 // 2 + 1) * W]
                if (g * (YG // 2) + yy // 2) % 2 == 0:
                    nc.scalar.mul(o2dst, psum_o2[:], -1.0)
                else:
                    nc.vector.tensor_scalar_mul(o2dst, psum_o2[:], -1.0)
                if q0 == 2:
                    y0 = ys[1] - 3
                    pr0 = y0 // 2
                    nc.sync.dma_start(
                        out[0:128, y0 : y0 + 4, :],
                        osb1[:].rearrange("p (y x) -> p y x", y=4),
                    )
                    nc.gpsimd.dma_start(
                        out2[:, :, pr0 : pr0 + 2, :].rearrange(
                            "j p r x -> (j p) (r x)"
                        ),
                        osb2[:],
                    )

    nc.compile()
    return nc


# ---------------- host-side helpers ----------------

def host_xn(c_u: np.ndarray) -> np.ndarray:
    """float64 reproduction of the reference displacement; returns xn [H, W] f32."""
    import math

    CUT = 16
    k = np.arange(1, CUT + 1, dtype=np.float64)
    i, j = np.meshgrid(k, k, indexing="ij")
    r = np.sqrt(i * i + j * j)
    e = (r < CUT + 0.5).astype(np.float64) / r
    x = np.linspace(0.0, 1.0, W, dtype=np.float64)
    s = np.sin(np.pi * x[:, None] * k[None, :])
    u = np.einsum("ij,xi,yj->yx", c_u.astype(np.float64) * e, s, s)
    Tw = 4.0 / (math.pi**3 * CUT**2 * math.log(CUT))
    dx = math.sqrt(Tw) * u * W
    xg = np.arange(W, dtype=np.float64)
    return np.clip(xg[None, :] - dx, 0.0, W - 1.0).astype(np.float32)


def _mask11(v: np.ndarray) -> np.ndarray:
    """Truncate fp32 mantissa to 10 explicit bits (fits fp16 significand)."""
    return (v.view(np.uint32) & np.uint32(0xFFFFE000)).view(np.float32)


def host_prep(img: np.ndarray, c_u: np.ndarray):
    """Build per-core input maps."""
    xn = host_xn(c_u)
    # band-coverage check: every tap P of column x must lie in window P//128
    m = np.floor(xn).astype(np.int64)
    xg = np.arange(W)[None, :]
    for tap in (m, np.minimum(m + 1, W - 1)):
        kk = tap // 128
        rel = xg - np.take(np.asarray(GS + [0]), kk)
        assert (rel >= 0).all() and (rel < GL).all(), "window coverage violated"
    planes = img.reshape(NPLANE, H, W)
    imgT_all = np.ascontiguousarray(
        planes.reshape(NPLANE, NCORES, NG, YG, W)
        .transpose(1, 2, 4, 3, 0)
        .astype(np.float16)
    )
    c4m = np.zeros((128, 128), np.float16)
    p = np.arange(128, dtype=np.float32)
    for a in range(4):
        c4m[32 * a + 0, :] = 1.0
        c4m[32 * a + 1, :] = 1.0
        c4m[32 * a + 2, :] = (-p).astype(np.float16)
    in_maps = []
    NF = (YPC // 4) * SW
    for core in range(NCORES):
        xns = np.zeros((12, NF), np.float16)
        for y in range(YPC):
            a, f = y % 4, y // 4
            yg = core * YPC + y
            for k in range(KBLK):
                seg = xn[yg, GS[k] : GS[k] + GL]
                xh = _mask11(seg)
                xm = (seg - xh).astype(np.float16)
                c0 = f * SW + GL * k
                xns[3 * a + 0, c0 : c0 + GL] = (xh - 128.0 * k).astype(np.float16)
                xns[3 * a + 1, c0 : c0 + GL] = xm
                xns[3 * a + 2, c0 : c0 + GL] = 1.0
        in_maps.append(
            {
                "imgT": imgT_all[core].reshape(NG, W, YG * NPLANE),
                "xn3": xns,
                "c4": c4m,
            }
        )
    return in_maps


def host_gather(outs: list) -> np.ndarray:
    """Assemble per-core 'out'/'out2' into [64, 3, H, W]."""
    full = np.empty((64, 3, H, W), np.float32)
    for core, om in enumerate(outs):
        o1 = om["out"].astype(np.float32)  # [128, YPC, W]
        # out2 [j, pl, pair, x] -> [pl, y=(pair*2+j), x]
        o2 = (
            om["out2"]
            .astype(np.float32)
            .transpose(1, 2, 0, 3)
            .reshape(64, YPC, W)
        )
        o = np.concatenate([o1, o2], axis=0)  # [NPLANE, YPC, W]
        full[:, :, core * YPC : (core + 1) * YPC, :] = o.reshape(64, 3, YPC, W)
    return full


# ---------------- harness entry point ----------------

_NC_CACHE = {}


def kernel(img: "np.ndarray", c_u: "np.ndarray", c_v: "np.ndarray") -> "np.ndarray":
    """Full-input entry: shard across 8 NeuronCores, run, reassemble."""
    img = np.ascontiguousarray(np.asarray(img, dtype=np.float32))
    c_u = np.asarray(c_u, dtype=np.float32)
    in_maps = host_prep(img, c_u)
    if "nc" not in _NC_CACHE:
        _NC_CACHE["nc"] = build_program(num_devices=NCORES)
    from concourse.bass_utils import run_bass_kernel_spmd

    res = run_bass_kernel_spmd(
        _NC_CACHE["nc"], in_maps, core_ids=list(range(NCORES)), trace=False
    )
    return host_gather(res.results)
